# revision 5
# baseline (speedup 1.0000x reference)
"""Multi-head attention with RoPE on 8 Trainium2 NeuronCores.

Problem: B=4, L=2048, D=1024, H=16 heads of dim 64, fp32, full (non-causal)
softmax attention with concatenated-halves RoPE on q and k.

Sharding: tensor-parallel over heads. Each of the 8 cores owns 2 heads:
 - computes q/k/v projections for its heads only (W_qkv column slice),
 - runs attention for its 2 heads x 4 batches,
 - computes a rank-128 partial of the output projection (W_proj row slice).
The host sums the 8 partial outputs (the only cross-core reduction).

On-core layout choices:
 - q, k are produced FEATURE-major ([head_dim, tokens]) directly by the QKV
   GEMM (weights pre-transposed/permuted on host), so the QK^T matmul needs
   no transposes. RoPE's even/odd feature split is pre-applied as a row
   permutation of W_q/W_k, so RoPE becomes 3 full-width DVE ops plus a
   32-partition-block swap done with SBUF->SBUF DMA.
 - scores are computed TRANSPOSED ([k_tokens, q_tokens]); softmax exp runs on
   ACT (scale folded into W_q on the host); the denominator comes free as an
   extra all-ones column appended to v in the p@v matmul.
 - v is produced feature-major then PE-transposed to token-major.
 - matmul operands are fp16 (PE streams 1 cycle/row vs 2 for fp32/fp32r);
   all accumulation stays fp32 in PSUM, softmax/rope/normalization math is
   fp32. Measured end-to-end relative error ~1e-3.
"""

import sys

for _p in ("/opt/trn_rl_repo",):
    if _p not in sys.path:
        sys.path.insert(0, _p)

import numpy as np
import concourse.bass as bass
import concourse.mybir as mybir
from concourse import bacc
from concourse.tile import TileContext
from concourse.bass_utils import run_bass_kernel_spmd
from concourse.masks import make_identity

F32 = mybir.dt.float32
F16 = mybir.dt.float16

B, L, D = 4, 2048, 1024
H, HD = 16, 64
NCORES = 8
HPC = H // NCORES  # 2 heads per core
TOK = B * L
BLK = 512  # gemm moving-dim block
QBLK = 512  # attention query block (one PSUM bank of fp32 output)
NBLK = L // BLK  # 4
NQB = L // QBLK  # 2
KT = D // 128  # 8 contraction tiles for the qkv projection
NKJ = L // 128  # 16 key tiles per batch
ROPE_BASE = 10000.0

Exp = mybir.ActivationFunctionType.Exp


def _build_program():
    nc = bacc.Bacc("TRN2", target_bir_lowering=False, debug=False)

    xt_d = nc.dram_tensor("xt", [D, TOK], F16, kind="ExternalInput")
    wqk_d = nc.dram_tensor("wqk", [D, 256], F16, kind="ExternalInput")
    wv_d = nc.dram_tensor("wv", [D, 128], F16, kind="ExternalInput")
    wp_d = nc.dram_tensor("wp", [128, D], F16, kind="ExternalInput")
    cc_d = nc.dram_tensor("cc", [128, L], F32, kind="ExternalInput")
    ssw_d = nc.dram_tensor("ssw", [128, L], F32, kind="ExternalInput")
    out_d = nc.dram_tensor("out", [B, D, L], F32, kind="ExternalOutput")

    with TileContext(nc) as tc:
        with (
            tc.tile_pool(name="singles", bufs=1) as singles,
            tc.tile_pool(name="xin", bufs=2) as xin,
            tc.tile_pool(name="batch", bufs=2) as batch,
            tc.tile_pool(name="rope", bufs=2) as rope,
            tc.tile_pool(name="pexp", bufs=4) as pexp,
            tc.tile_pool(name="norm", bufs=4) as norm,
            tc.tile_pool(name="outp", bufs=4) as outp,
            tc.tile_pool(name="ps_g", bufs=2, space="PSUM") as ps_g,
            tc.tile_pool(name="ps_s", bufs=4, space="PSUM") as ps_s,
            tc.tile_pool(name="ps_o", bufs=2, space="PSUM") as ps_o,
        ):
            # resident weights / tables
            wqk_sb = singles.tile([128, KT, 256], F16, tag="wqk")
            nc.sync.dma_start(
                out=wqk_sb[:], in_=wqk_d[:, :].rearrange("(k p) e -> p k e", p=128)
            )
            wv_sb = singles.tile([128, KT, 128], F16, tag="wv")
            nc.sync.dma_start(
                out=wv_sb[:], in_=wv_d[:, :].rearrange("(k p) e -> p k e", p=128)
            )
            wp_sb = singles.tile([128, D], F16, tag="wp")
            nc.sync.dma_start(out=wp_sb[:], in_=wp_d[:, :])
            cc_sb = singles.tile([128, L], F32, tag="cc")
            nc.sync.dma_start(out=cc_sb[:], in_=cc_d[:, :])
            ssw_sb = singles.tile([128, L], F32, tag="ssw")
            nc.sync.dma_start(out=ssw_sb[:], in_=ssw_d[:, :])
            ident = singles.tile([128, 128], F16, tag="ident")
            make_identity(nc, ident[:])

            for b in range(B):
                q_ro = batch.tile([128, L], F16, tag="qro")
                k_ro = batch.tile([128, L], F16, tag="kro")
                v_fm = batch.tile([128, L], F16, tag="vfm")
                v0 = batch.tile([128, NKJ, 65], F16, tag="v0")
                v1 = batch.tile([128, NKJ, 65], F16, tag="v1")
                ao = batch.tile([128, L], F16, tag="ao")

                # ---- phase 1: qkv projection + rope ----
                for blk in range(NBLK):
                    ts = slice(blk * BLK, (blk + 1) * BLK)
                    x_t = xin.tile([128, KT, BLK], F16, tag="x")
                    nc.sync.dma_start(
                        out=x_t[:],
                        in_=xt_d[:, b * L + blk * BLK : b * L + (blk + 1) * BLK]
                        .rearrange("(k p) t -> p k t", p=128),
                    )
                    for wcol, dst in ((0, q_ro), (128, k_ro)):
                        ps = ps_g.tile([128, BLK], F32, tag="g")
                        for kd in range(KT):
                            nc.tensor.matmul(
                                ps[:],
                                wqk_sb[:, kd, wcol : wcol + 128],
                                x_t[:, kd, :],
                                start=(kd == 0),
                                stop=(kd == KT - 1),
                            )
                        tmp_c = rope.tile([128, BLK], F32, tag="tc")
                        nc.vector.tensor_mul(tmp_c[:], ps[:], cc_sb[:, ts])
                        tmp_s = rope.tile([128, BLK], F32, tag="tsn")
                        nc.vector.tensor_mul(tmp_s[:], ps[:], ssw_sb[:, ts])
                        tmp_w = rope.tile([128, BLK], F32, tag="tw")
                        for a, bb in ((0, 32), (32, 0), (64, 96), (96, 64)):
                            nc.sync.dma_start(
                                out=tmp_w[a : a + 32, :], in_=tmp_s[bb : bb + 32, :]
                            )
                        nc.vector.tensor_add(dst[:, ts], tmp_c[:], tmp_w[:])

                    psv = ps_g.tile([128, BLK], F32, tag="g")
                    for kd in range(KT):
                        nc.tensor.matmul(
                            psv[:],
                            wv_sb[:, kd, :],
                            x_t[:, kd, :],
                            start=(kd == 0),
                            stop=(kd == KT - 1),
                        )
                    nc.scalar.copy(v_fm[:, ts], psv[:])

                # v: feature-major -> token-major (+ ones column for softmax sums)
                nc.vector.memset(v0[:, :, 64], 1.0)
                nc.vector.memset(v1[:, :, 64], 1.0)
                for tt in range(NKJ):
                    pst = ps_g.tile([128, 128], F16, tag="g")
                    nc.tensor.transpose(
                        pst[:], v_fm[:, tt * 128 : (tt + 1) * 128], ident[:]
                    )
                    nc.scalar.copy(v0[:, tt, 0:64], pst[:, 0:64])
                    nc.scalar.copy(v1[:, tt, 0:64], pst[:, 64:128])

                # ---- phase 2: attention (scores transposed; per-head serial) ----
                for qi in range(NQB):
                    qs = slice(qi * QBLK, (qi + 1) * QBLK)
                    o0 = ps_o.tile([65, QBLK], F32, tag="o")
                    o1 = ps_o.tile([65, QBLK], F32, tag="o")
                    for kj in range(NKJ):
                        ks = slice(kj * 128, (kj + 1) * 128)
                        p0 = pexp.tile([128, QBLK], F16, tag="p")
                        p1 = pexp.tile([128, QBLK], F16, tag="p")
                        for hb, ph in ((0, p0), (64, p1)):
                            s_ps = ps_s.tile([128, QBLK], F32, tag="s")
                            nc.tensor.matmul(
                                s_ps[:],
                                k_ro[hb : hb + 64, ks],
                                q_ro[hb : hb + 64, qs],
                                start=True,
                                stop=True,
                            )
                            nc.scalar.activation(ph[:], s_ps[:], Exp)
                        nc.tensor.matmul(
                            o0[:], v0[:, kj, :], p0[:],
                            start=(kj == 0), stop=(kj == NKJ - 1),
                        )
                        nc.tensor.matmul(
                            o1[:], v1[:, kj, :], p1[:],
                            start=(kj == 0), stop=(kj == NKJ - 1),
                        )
                    for o_ps, base in ((o0, 0), (o1, 64)):
                        r = norm.tile([1, QBLK], F32, tag="r")
                        nc.vector.reciprocal(r[:], o_ps[64:65, :])
                        rb = norm.tile([64, QBLK], F32, tag="rb")
                        nc.gpsimd.partition_broadcast(rb[:], r[:])
                        nc.vector.tensor_mul(
                            ao[base : base + 64, qs], o_ps[0:64, :], rb[:]
                        )

                # ---- phase 3: output projection (partial over this core's 128 dims)
                for blk in range(NBLK):
                    ts = slice(blk * BLK, (blk + 1) * BLK)
                    for e in range(D // 128):
                        psf = ps_g.tile([128, BLK], F32, tag="g")
                        nc.tensor.matmul(
                            psf[:],
                            wp_sb[:, e * 128 : (e + 1) * 128],
                            ao[:, ts],
                            start=True,
                            stop=True,
                        )
                        o_sb = outp.tile([128, BLK], F32, tag="os")
                        nc.vector.tensor_copy(o_sb[:], psf[:])
                        nc.sync.dma_start(
                            out=out_d[b, e * 128 : (e + 1) * 128, ts], in_=o_sb[:]
                        )

    nc.compile()
    return nc


_PROGRAM = None


def _program():
    global _PROGRAM
    if _PROGRAM is None:
        _PROGRAM = _build_program()
    return _PROGRAM


def _rope_tables():
    f = np.arange(32, dtype=np.float64)
    inv = ROPE_BASE ** (-2.0 * f / HD)
    t = np.arange(L, dtype=np.float64)
    ang = np.outer(inv, t)  # [32, L]
    cosT = np.cos(ang)
    sinT = np.sin(ang)
    cc = np.tile(cosT, (4, 1)).astype(np.float32)  # [128, L]
    ssw = np.concatenate([sinT, -sinT, sinT, -sinT], axis=0).astype(np.float32)
    return cc, ssw


def _prep_in_maps(x, W_qkv, W_proj):
    xt = np.ascontiguousarray(x.reshape(TOK, D).T).astype(np.float16)
    cc, ssw = _rope_tables()
    scale = HD**-0.5

    evens = np.arange(0, HD, 2)
    odds = np.arange(1, HD, 2)
    in_maps = []
    for c in range(NCORES):
        h0, h1 = HPC * c, HPC * c + 1
        rows_pair = np.concatenate(
            [h0 * HD + evens, h0 * HD + odds, h1 * HD + evens, h1 * HD + odds]
        )
        wq = (W_qkv[rows_pair, :].astype(np.float64) * scale).T  # [D, 128]
        wk = W_qkv[D + rows_pair, :].T  # [D, 128]
        wqk = np.concatenate([wq, wk], axis=1).astype(np.float16)
        rows_v = np.concatenate(
            [2 * D + h0 * HD + np.arange(HD), 2 * D + h1 * HD + np.arange(HD)]
        )
        wv = np.ascontiguousarray(W_qkv[rows_v, :].T).astype(np.float16)  # [D, 128]
        d_rows = np.concatenate([h0 * HD + np.arange(HD), h1 * HD + np.arange(HD)])
        wp = np.ascontiguousarray(W_proj[:, d_rows].T).astype(np.float16)  # [128, D]
        in_maps.append(
            {"xt": xt, "wqk": wqk, "wv": wv, "wp": wp, "cc": cc, "ssw": ssw}
        )
    return in_maps


def run(x, W_qkv, W_proj, trace=False):
    nc = _program()
    in_maps = _prep_in_maps(np.asarray(x), np.asarray(W_qkv), np.asarray(W_proj))
    res = run_bass_kernel_spmd(
        nc, in_maps, core_ids=list(range(NCORES)), trace=trace
    )
    acc = res.results[0]["out"].astype(np.float64)
    for c in range(1, NCORES):
        acc += res.results[c]["out"]
    full = np.transpose(acc, (0, 2, 1)).astype(np.float32)  # [B, L, D]
    return full, res


def kernel(x, W_qkv, W_proj):
    out, _ = run(x, W_qkv, W_proj, trace=False)
    return out


# revision 16
# speedup vs baseline: 1.5114x; 1.5114x over previous
"""Multi-head attention with RoPE on 8 Trainium2 NeuronCores.

Problem: B=4, L=2048, D=1024, H=16 heads of dim 64, fp32, full (non-causal)
softmax attention with concatenated-halves RoPE on q and k.

Sharding: tensor-parallel over heads. Each of the 8 cores owns 2 heads:
 - computes q/k/v projections for its heads only (W_qkv column slice),
 - runs attention for its 2 heads x 4 batches,
 - computes a rank-128 partial of the output projection (W_proj row slice).
The host sums the 8 partial outputs (the only cross-core reduction).

On-core layout choices:
 - q, k are produced FEATURE-major ([head_dim, tokens]) directly by the QKV
   GEMM (weights pre-transposed/permuted on host), so the QK^T matmul needs
   no transposes. RoPE's even/odd feature split is pre-applied as a row
   permutation of W_q/W_k, so RoPE becomes 3 full-width DVE ops plus a
   32-partition-block swap done with SBUF->SBUF DMA.
 - scores are computed TRANSPOSED ([k_tokens, q_tokens]); softmax exp runs on
   ACT (scale folded into W_q on the host); the denominator comes free as an
   extra all-ones column appended to v in the p@v matmul.
 - v is produced feature-major then PE-transposed to token-major.
 - matmul operands are fp16 (PE streams 1 cycle/row vs 2 for fp32/fp32r);
   all accumulation stays fp32 in PSUM, softmax/rope/normalization math is
   fp32. Measured end-to-end relative error ~8e-4.
 - emission is software-pipelined: phase1 of batch b+1 and the output
   projection of batch b are emitted between the attention chunks of batch
   b so the Tile scheduler interleaves them into ACT-bound gaps.
"""

import sys

for _p in ("/opt/trn_rl_repo",):
    if _p not in sys.path:
        sys.path.insert(0, _p)

import numpy as np
import concourse.bass as bass
import concourse.mybir as mybir
from concourse import bacc
from concourse.tile import TileContext
from concourse.bass_utils import run_bass_kernel_spmd
from concourse.masks import make_identity

F32 = mybir.dt.float32
F16 = mybir.dt.float16

B, L, D = 4, 2048, 1024
H, HD = 16, 64
NCORES = 8
HPC = H // NCORES  # 2 heads per core
TOK = B * L
BLK = 512  # gemm moving-dim block
QBLK = 512  # attention query block (one PSUM bank of fp32 output)
NBLK = L // BLK  # 4
NQB = L // QBLK  # 4
KT = D // 128  # 8 contraction tiles for the qkv projection
NKJ = L // 128  # 16 key tiles per batch
ROPE_BASE = 10000.0

Exp = mybir.ActivationFunctionType.Exp


class _Ctx:
    pass


def _build_program():
    nc = bacc.Bacc("TRN2", target_bir_lowering=False, debug=False)

    c = _Ctx()
    c.nc = nc
    c.xt_d = nc.dram_tensor("xt", [D, TOK], F16, kind="ExternalInput")
    c.wqk_d = nc.dram_tensor("wqk", [D, 256], F16, kind="ExternalInput")
    c.wv_d = nc.dram_tensor("wv", [D, 128], F16, kind="ExternalInput")
    c.wp_d = nc.dram_tensor("wp", [128, D], F16, kind="ExternalInput")
    c.cc_d = nc.dram_tensor("cc", [128, L], F32, kind="ExternalInput")
    c.ssw_d = nc.dram_tensor("ssw", [128, L], F32, kind="ExternalInput")
    c.out_d = nc.dram_tensor("out", [B, D, L], F32, kind="ExternalOutput")

    with TileContext(nc) as tc:
        with (
            tc.tile_pool(name="singles", bufs=1) as singles,
            tc.tile_pool(name="xin", bufs=2) as xin,
            tc.tile_pool(name="batch", bufs=2) as batch,
            tc.tile_pool(name="rope", bufs=2) as rope,
            tc.tile_pool(name="pexp", bufs=4) as pexp,
            tc.tile_pool(name="norm", bufs=4) as norm,
            tc.tile_pool(name="outp", bufs=4) as outp,
            tc.tile_pool(name="ps_g", bufs=2, space="PSUM") as ps_g,
            tc.tile_pool(name="ps_s", bufs=2, space="PSUM") as ps_s,
            tc.tile_pool(name="ps_o", bufs=2, space="PSUM") as ps_o,
        ):
            c.xin, c.batch, c.rope = xin, batch, rope
            c.pexp, c.norm, c.outp = pexp, norm, outp
            c.ps_g, c.ps_s, c.ps_o = ps_g, ps_s, ps_o

            # resident weights / tables
            c.wqk_sb = singles.tile([128, KT, 256], F16, tag="wqk")
            nc.sync.dma_start(
                out=c.wqk_sb[:], in_=c.wqk_d[:, :].rearrange("(k p) e -> p k e", p=128)
            )
            c.wv_sb = singles.tile([128, KT, 128], F16, tag="wv")
            nc.sync.dma_start(
                out=c.wv_sb[:], in_=c.wv_d[:, :].rearrange("(k p) e -> p k e", p=128)
            )
            c.wp_sb = singles.tile([128, D], F16, tag="wp")
            nc.sync.dma_start(out=c.wp_sb[:], in_=c.wp_d[:, :])
            c.cc_sb = singles.tile([128, L], F32, tag="cc")
            nc.sync.dma_start(out=c.cc_sb[:], in_=c.cc_d[:, :])
            c.ssw_sb = singles.tile([128, L], F32, tag="ssw")
            nc.sync.dma_start(out=c.ssw_sb[:], in_=c.ssw_d[:, :])
            c.ident = singles.tile([128, 128], F16, tag="ident")
            make_identity(nc, c.ident[:])

            c.bt = {}

            # software-pipelined emission
            for blk in range(NBLK):
                _phase1_chunk(c, 0, blk)
            _vtrans(c, 0)
            for b in range(B):
                for qi in range(NQB):
                    _phase2_chunk(c, b, qi)
                    if b + 1 < B:
                        _phase1_chunk(c, b + 1, qi)
                        if qi == NQB - 1:
                            _vtrans(c, b + 1)
                    if qi >= 1:
                        _phase3_chunk(c, b, qi - 1)
                _phase3_chunk(c, b, NQB - 1)

    nc.compile()
    return nc


def _tiles(c, b):
    if b not in c.bt:
        t = _Ctx()
        t.q_ro = c.batch.tile([128, L], F16, tag="qro")
        t.k_ro = c.batch.tile([128, L], F16, tag="kro")
        t.v_fm = c.batch.tile([128, L], F16, tag="vfm")
        t.v0 = c.batch.tile([128, NKJ, 65], F16, tag="v0")
        t.v1 = c.batch.tile([128, NKJ, 65], F16, tag="v1")
        t.ao = c.batch.tile([128, L], F16, tag="ao")
        t.x_t = None
        c.bt[b] = t
    return c.bt[b]


def _phase1_chunk(c, b, blk):
    nc = c.nc
    t = _tiles(c, b)
    ts = slice(blk * BLK, (blk + 1) * BLK)
    if blk == 0:
        t.x_t = c.xin.tile([128, KT, L], F16, tag="x")
        nc.sync.dma_start(
            out=t.x_t[:],
            in_=c.xt_d[:, b * L : (b + 1) * L].rearrange("(k p) t -> p k t", p=128),
        )
    for wcol, dst in ((0, t.q_ro), (128, t.k_ro)):
        ps = c.ps_g.tile([128, BLK], F32, tag="g")
        for kd in range(KT):
            nc.tensor.matmul(
                ps[:],
                c.wqk_sb[:, kd, wcol : wcol + 128],
                t.x_t[:, kd, ts],
                start=(kd == 0),
                stop=(kd == KT - 1),
            )
        tmp_c = c.rope.tile([128, BLK], F32, tag="tc")
        nc.vector.tensor_mul(tmp_c[:], ps[:], c.cc_sb[:, ts])
        tmp_s = c.rope.tile([128, BLK], F32, tag="tsn")
        nc.vector.tensor_mul(tmp_s[:], ps[:], c.ssw_sb[:, ts])
        tmp_w = c.rope.tile([128, BLK], F32, tag="tw")
        for a, bb in ((0, 32), (32, 0), (64, 96), (96, 64)):
            nc.sync.dma_start(out=tmp_w[a : a + 32, :], in_=tmp_s[bb : bb + 32, :])
        nc.vector.tensor_add(dst[:, ts], tmp_c[:], tmp_w[:])

    psv = c.ps_g.tile([128, BLK], F32, tag="g")
    for kd in range(KT):
        nc.tensor.matmul(
            psv[:],
            c.wv_sb[:, kd, :],
            t.x_t[:, kd, ts],
            start=(kd == 0),
            stop=(kd == KT - 1),
        )
    nc.scalar.copy(t.v_fm[:, ts], psv[:])


def _vtrans(c, b):
    nc = c.nc
    t = _tiles(c, b)
    nc.vector.memset(t.v0[:, :, 64], 1.0)
    nc.vector.memset(t.v1[:, :, 64], 1.0)
    for tt in range(NKJ):
        pst = c.ps_g.tile([128, 128], F16, tag="g")
        nc.tensor.transpose(pst[:], t.v_fm[:, tt * 128 : (tt + 1) * 128], c.ident[:])
        nc.scalar.copy(t.v0[:, tt, 0:64], pst[:, 0:64])
        nc.scalar.copy(t.v1[:, tt, 0:64], pst[:, 64:128])


def _phase2_chunk(c, b, qi):
    nc = c.nc
    t = _tiles(c, b)
    qs = slice(qi * QBLK, (qi + 1) * QBLK)
    o0 = c.ps_o.tile([65, QBLK], F32, tag="o")
    o1 = c.ps_o.tile([65, QBLK], F32, tag="o")
    for kj in range(NKJ):
        ks = slice(kj * 128, (kj + 1) * 128)
        s_ps = c.ps_s.tile([128, 2 * QBLK], F32, tag="s")
        nc.tensor.matmul(
            s_ps[:, 0:QBLK], t.k_ro[0:64, ks], t.q_ro[0:64, qs],
            start=True, stop=True,
        )
        nc.tensor.matmul(
            s_ps[:, QBLK : 2 * QBLK],
            t.k_ro[64:128, ks],
            t.q_ro[64:128, qs],
            start=True,
            stop=True,
            tile_position=(64, 0),
        )
        p = c.pexp.tile([128, 2 * QBLK], F16, tag="p")
        nc.scalar.activation(p[:], s_ps[:], Exp)
        nc.tensor.matmul(
            o0[:], t.v0[:, kj, :], p[:, 0:QBLK],
            start=(kj == 0), stop=(kj == NKJ - 1),
        )
        nc.tensor.matmul(
            o1[:], t.v1[:, kj, :], p[:, QBLK : 2 * QBLK],
            start=(kj == 0), stop=(kj == NKJ - 1),
        )
    # early copies release the o-psum banks; recip/broadcast/multiply run
    # off the PE critical path.
    rb_full = c.norm.tile([128, QBLK], F32, tag="rbf")
    for o_ps, base in ((o0, 0), (o1, 64)):
        nc.scalar.copy(t.ao[base : base + 64, qs], o_ps[0:64, :])
        stg = c.norm.tile([1, QBLK], F32, tag="stg")
        nc.vector.tensor_copy(stg[:], o_ps[64:65, :])
        r = c.norm.tile([1, QBLK], F32, tag="r")
        nc.vector.reciprocal(r[:], stg[:])
        if base == 0:
            nc.gpsimd.partition_broadcast(rb_full[0:64, :], r[:])
        else:
            rb1 = c.norm.tile([64, QBLK], F32, tag="rb")
            nc.gpsimd.partition_broadcast(rb1[:], r[:])
            nc.vector.tensor_copy(rb_full[64:128, :], rb1[:])
    nc.vector.tensor_mul(t.ao[:, qs], t.ao[:, qs], rb_full[:])


def _phase3_chunk(c, b, blk):
    nc = c.nc
    t = _tiles(c, b)
    ts = slice(blk * BLK, (blk + 1) * BLK)
    for e in range(D // 128):
        psf = c.ps_g.tile([128, BLK], F32, tag="g")
        nc.tensor.matmul(
            psf[:],
            c.wp_sb[:, e * 128 : (e + 1) * 128],
            t.ao[:, ts],
            start=True,
            stop=True,
        )
        o_sb = c.outp.tile([128, BLK], F32, tag="os")
        nc.vector.tensor_copy(o_sb[:], psf[:])
        nc.sync.dma_start(out=c.out_d[b, e * 128 : (e + 1) * 128, ts], in_=o_sb[:])


_PROGRAM = None


def _program():
    global _PROGRAM
    if _PROGRAM is None:
        _PROGRAM = _build_program()
    return _PROGRAM


def _rope_tables():
    f = np.arange(32, dtype=np.float64)
    inv = ROPE_BASE ** (-2.0 * f / HD)
    t = np.arange(L, dtype=np.float64)
    ang = np.outer(inv, t)  # [32, L]
    cosT = np.cos(ang)
    sinT = np.sin(ang)
    cc = np.tile(cosT, (4, 1)).astype(np.float32)  # [128, L]
    ssw = np.concatenate([sinT, -sinT, sinT, -sinT], axis=0).astype(np.float32)
    return cc, ssw


def _prep_in_maps(x, W_qkv, W_proj):
    xt = np.ascontiguousarray(x.reshape(TOK, D).T).astype(np.float16)
    cc, ssw = _rope_tables()
    scale = HD**-0.5

    evens = np.arange(0, HD, 2)
    odds = np.arange(1, HD, 2)
    in_maps = []
    for c in range(NCORES):
        h0, h1 = HPC * c, HPC * c + 1
        rows_pair = np.concatenate(
            [h0 * HD + evens, h0 * HD + odds, h1 * HD + evens, h1 * HD + odds]
        )
        wq = (W_qkv[rows_pair, :].astype(np.float64) * scale).T  # [D, 128]
        wk = W_qkv[D + rows_pair, :].T  # [D, 128]
        wqk = np.concatenate([wq, wk], axis=1).astype(np.float16)
        rows_v = np.concatenate(
            [2 * D + h0 * HD + np.arange(HD), 2 * D + h1 * HD + np.arange(HD)]
        )
        wv = np.ascontiguousarray(W_qkv[rows_v, :].T).astype(np.float16)  # [D, 128]
        d_rows = np.concatenate([h0 * HD + np.arange(HD), h1 * HD + np.arange(HD)])
        wp = np.ascontiguousarray(W_proj[:, d_rows].T).astype(np.float16)  # [128, D]
        in_maps.append(
            {"xt": xt, "wqk": wqk, "wv": wv, "wp": wp, "cc": cc, "ssw": ssw}
        )
    return in_maps


def run(x, W_qkv, W_proj, trace=False):
    nc = _program()
    in_maps = _prep_in_maps(np.asarray(x), np.asarray(W_qkv), np.asarray(W_proj))
    res = run_bass_kernel_spmd(
        nc, in_maps, core_ids=list(range(NCORES)), trace=trace
    )
    acc = res.results[0]["out"].astype(np.float64)
    for c in range(1, NCORES):
        acc += res.results[c]["out"]
    full = np.transpose(acc, (0, 2, 1)).astype(np.float32)  # [B, L, D]
    return full, res


def kernel(x, W_qkv, W_proj):
    out, _ = run(x, W_qkv, W_proj, trace=False)
    return out


# revision 17
# speedup vs baseline: 1.7582x; 1.1633x over previous
"""Multi-head attention with RoPE on 8 Trainium2 NeuronCores.

Problem: B=4, L=2048, D=1024, H=16 heads of dim 64, fp32, full (non-causal)
softmax attention with concatenated-halves RoPE on q and k.

Sharding: tensor-parallel over heads. Each of the 8 cores owns 2 heads:
 - computes q/k/v projections for its heads only (W_qkv column slice),
 - runs attention for its 2 heads x 4 batches,
 - computes a rank-128 partial of the output projection (W_proj row slice).
The host sums the 8 partial outputs (the only cross-core reduction).

On-core layout choices:
 - q, k are produced FEATURE-major ([head_dim, tokens]) directly by the QKV
   GEMM (weights pre-transposed/permuted on host), so the QK^T matmul needs
   no transposes. RoPE's even/odd feature split is pre-applied as a row
   permutation of W_q/W_k, so RoPE becomes 3 full-width DVE ops plus a
   32-partition-block swap done with SBUF->SBUF DMA.
 - scores are computed TRANSPOSED ([k_tokens, q_tokens]); softmax exp runs on
   ACT (scale folded into W_q on the host); the denominator comes free as an
   extra all-ones column appended to v in the p@v matmul.
 - v is produced feature-major then PE-transposed to token-major.
 - matmul operands are fp16 (PE streams 1 cycle/row vs 2 for fp32/fp32r);
   all accumulation stays fp32 in PSUM, softmax/rope/normalization math is
   fp32. Measured end-to-end relative error ~8e-4.
 - emission is software-pipelined: phase1 of batch b+1 and the output
   projection of batch b are emitted between the attention chunks of batch
   b so the Tile scheduler interleaves them into ACT-bound gaps.
"""

import sys

for _p in ("/opt/trn_rl_repo",):
    if _p not in sys.path:
        sys.path.insert(0, _p)

import numpy as np
import concourse.bass as bass
import concourse.mybir as mybir
from concourse import bacc
from concourse.tile import TileContext
from concourse.bass_utils import run_bass_kernel_spmd
from concourse.masks import make_identity

F32 = mybir.dt.float32
F16 = mybir.dt.float16

B, L, D = 4, 2048, 1024
H, HD = 16, 64
NCORES = 8
HPC = H // NCORES  # 2 heads per core
TOK = B * L
BLK = 512  # gemm moving-dim block
QBLK = 512  # attention query block (one PSUM bank of fp32 output)
NBLK = L // BLK  # 4
NQB = L // QBLK  # 4
KT = D // 128  # 8 contraction tiles for the qkv projection
NKJ = L // 128  # 16 key tiles per batch
ROPE_BASE = 10000.0

Exp = mybir.ActivationFunctionType.Exp


class _Ctx:
    pass


def _build_program():
    nc = bacc.Bacc("TRN2", target_bir_lowering=False, debug=False)

    c = _Ctx()
    c.nc = nc
    c.xt_d = nc.dram_tensor("xt", [D, TOK], F16, kind="ExternalInput")
    c.wqk_d = nc.dram_tensor("wqk", [D, 256], F16, kind="ExternalInput")
    c.wv_d = nc.dram_tensor("wv", [D, 128], F16, kind="ExternalInput")
    c.wp_d = nc.dram_tensor("wp", [128, D], F16, kind="ExternalInput")
    c.cc_d = nc.dram_tensor("cc", [128, L], F32, kind="ExternalInput")
    c.ssw_d = nc.dram_tensor("ssw", [128, L], F32, kind="ExternalInput")
    c.out_d = nc.dram_tensor("out", [B, D, L], F32, kind="ExternalOutput")

    with TileContext(nc) as tc:
        with (
            tc.tile_pool(name="singles", bufs=1) as singles,
            tc.tile_pool(name="xin", bufs=2) as xin,
            tc.tile_pool(name="batch", bufs=2) as batch,
            tc.tile_pool(name="rope", bufs=2) as rope,
            tc.tile_pool(name="pexp", bufs=4) as pexp,
            tc.tile_pool(name="norm", bufs=4) as norm,
            tc.tile_pool(name="outp", bufs=4) as outp,
            tc.tile_pool(name="ps_g", bufs=2, space="PSUM") as ps_g,
            tc.tile_pool(name="ps_s", bufs=2, space="PSUM") as ps_s,
            tc.tile_pool(name="ps_o", bufs=2, space="PSUM") as ps_o,
        ):
            c.xin, c.batch, c.rope = xin, batch, rope
            c.pexp, c.norm, c.outp = pexp, norm, outp
            c.ps_g, c.ps_s, c.ps_o = ps_g, ps_s, ps_o

            # resident weights / tables
            c.wqk_sb = singles.tile([128, KT, 256], F16, tag="wqk")
            nc.sync.dma_start(
                out=c.wqk_sb[:], in_=c.wqk_d[:, :].rearrange("(k p) e -> p k e", p=128)
            )
            c.wv_sb = singles.tile([128, KT, 128], F16, tag="wv")
            nc.sync.dma_start(
                out=c.wv_sb[:], in_=c.wv_d[:, :].rearrange("(k p) e -> p k e", p=128)
            )
            c.wp_sb = singles.tile([128, D], F16, tag="wp")
            nc.sync.dma_start(out=c.wp_sb[:], in_=c.wp_d[:, :])
            c.cc_sb = singles.tile([128, L], F32, tag="cc")
            nc.sync.dma_start(out=c.cc_sb[:], in_=c.cc_d[:, :])
            c.ssw_sb = singles.tile([128, L], F32, tag="ssw")
            nc.sync.dma_start(out=c.ssw_sb[:], in_=c.ssw_d[:, :])
            c.ident = singles.tile([128, 128], F16, tag="ident")
            make_identity(nc, c.ident[:])

            c.bt = {}

            # software-pipelined emission
            for blk in range(NBLK):
                _phase1_chunk(c, 0, blk)
            _vtrans(c, 0)
            for b in range(B):
                for qi in range(NQB):
                    _phase2_chunk(c, b, qi)
                    if b + 1 < B:
                        _phase1_chunk(c, b + 1, qi)
                        if qi == NQB - 1:
                            _vtrans(c, b + 1)
                    if qi >= 1:
                        _phase3_chunk(c, b, qi - 1)
                _phase3_chunk(c, b, NQB - 1)

    nc.compile()
    return nc


def _tiles(c, b):
    if b not in c.bt:
        t = _Ctx()
        t.q_ro = c.batch.tile([128, L], F16, tag="qro")
        t.k_ro = c.batch.tile([128, L], F16, tag="kro")
        t.v_fm = c.batch.tile([128, L], F16, tag="vfm")
        t.v0 = c.batch.tile([128, NKJ, 65], F16, tag="v0")
        t.v1 = c.batch.tile([128, NKJ, 65], F16, tag="v1")
        t.ao = c.batch.tile([128, L], F16, tag="ao")
        t.x_t = None
        c.bt[b] = t
    return c.bt[b]


def _phase1_chunk(c, b, blk):
    nc = c.nc
    t = _tiles(c, b)
    ts = slice(blk * BLK, (blk + 1) * BLK)
    if blk == 0:
        t.x_t = c.xin.tile([128, KT, L], F16, tag="x")
        nc.sync.dma_start(
            out=t.x_t[:],
            in_=c.xt_d[:, b * L : (b + 1) * L].rearrange("(k p) t -> p k t", p=128),
        )
    for wcol, dst in ((0, t.q_ro), (128, t.k_ro)):
        ps = c.ps_g.tile([128, BLK], F32, tag="g")
        for kd in range(KT):
            nc.tensor.matmul(
                ps[:],
                c.wqk_sb[:, kd, wcol : wcol + 128],
                t.x_t[:, kd, ts],
                start=(kd == 0),
                stop=(kd == KT - 1),
            )
        tmp_c = c.rope.tile([128, BLK], F32, tag="tc")
        nc.vector.tensor_mul(tmp_c[:], ps[:], c.cc_sb[:, ts])
        tmp_s = c.rope.tile([128, BLK], F32, tag="tsn")
        nc.vector.tensor_mul(tmp_s[:], ps[:], c.ssw_sb[:, ts])
        tmp_w = c.rope.tile([128, BLK], F32, tag="tw")
        for a, bb in ((0, 32), (32, 0), (64, 96), (96, 64)):
            nc.sync.dma_start(out=tmp_w[a : a + 32, :], in_=tmp_s[bb : bb + 32, :])
        nc.vector.tensor_add(dst[:, ts], tmp_c[:], tmp_w[:])

    psv = c.ps_g.tile([128, BLK], F32, tag="g")
    for kd in range(KT):
        nc.tensor.matmul(
            psv[:],
            c.wv_sb[:, kd, :],
            t.x_t[:, kd, ts],
            start=(kd == 0),
            stop=(kd == KT - 1),
        )
    nc.scalar.copy(t.v_fm[:, ts], psv[:])


def _vtrans(c, b):
    nc = c.nc
    t = _tiles(c, b)
    nc.vector.memset(t.v0[:, :, 64], 1.0)
    nc.vector.memset(t.v1[:, :, 64], 1.0)
    for tt in range(NKJ):
        pst = c.ps_g.tile([128, 128], F16, tag="g")
        nc.tensor.transpose(pst[:], t.v_fm[:, tt * 128 : (tt + 1) * 128], c.ident[:])
        nc.scalar.copy(t.v0[:, tt, 0:64], pst[:, 0:64])
        nc.scalar.copy(t.v1[:, tt, 0:64], pst[:, 64:128])


def _phase2_chunk(c, b, qi):
    nc = c.nc
    t = _tiles(c, b)
    qs = slice(qi * QBLK, (qi + 1) * QBLK)
    o0 = c.ps_o.tile([65, QBLK], F32, tag="o")
    o1 = c.ps_o.tile([65, QBLK], F32, tag="o")
    for kj in range(NKJ):
        ks = slice(kj * 128, (kj + 1) * 128)
        s_ps = c.ps_s.tile([128, 2 * QBLK], F32, tag="s")
        nc.tensor.matmul(
            s_ps[:, 0:QBLK], t.k_ro[0:64, ks], t.q_ro[0:64, qs],
            start=True, stop=True,
        )
        nc.tensor.matmul(
            s_ps[:, QBLK : 2 * QBLK],
            t.k_ro[64:128, ks],
            t.q_ro[64:128, qs],
            start=True,
            stop=True,
            tile_position=(64, 0),
        )
        p = c.pexp.tile([128, 2 * QBLK], F16, tag="p")
        nc.scalar.activation(p[:], s_ps[:], Exp)
        nc.tensor.matmul(
            o0[:], t.v0[:, kj, :], p[:, 0:QBLK],
            start=(kj == 0), stop=(kj == NKJ - 1),
        )
        nc.tensor.matmul(
            o1[:], t.v1[:, kj, :], p[:, QBLK : 2 * QBLK],
            start=(kj == 0), stop=(kj == NKJ - 1),
        )
    # early copies release the o-psum banks; recip/broadcast/multiply run
    # off the PE critical path.
    rb_full = c.norm.tile([128, QBLK], F32, tag="rbf")
    for o_ps, base in ((o0, 0), (o1, 64)):
        nc.scalar.copy(t.ao[base : base + 64, qs], o_ps[0:64, :])
        stg = c.norm.tile([1, QBLK], F32, tag="stg")
        nc.scalar.copy(stg[:], o_ps[64:65, :])
        r = c.norm.tile([1, QBLK], F32, tag="r")
        nc.vector.reciprocal_approx_fast(r[:], stg[:])
        if base == 0:
            nc.gpsimd.partition_broadcast(rb_full[0:64, :], r[:])
        else:
            rb1 = c.norm.tile([64, QBLK], F32, tag="rb")
            nc.gpsimd.partition_broadcast(rb1[:], r[:])
            nc.vector.tensor_copy(rb_full[64:128, :], rb1[:])
    nc.vector.tensor_mul(t.ao[:, qs], t.ao[:, qs], rb_full[:])


def _phase3_chunk(c, b, blk):
    nc = c.nc
    t = _tiles(c, b)
    ts = slice(blk * BLK, (blk + 1) * BLK)
    for e in range(D // 128):
        psf = c.ps_g.tile([128, BLK], F32, tag="g")
        nc.tensor.matmul(
            psf[:],
            c.wp_sb[:, e * 128 : (e + 1) * 128],
            t.ao[:, ts],
            start=True,
            stop=True,
        )
        o_sb = c.outp.tile([128, BLK], F32, tag="os")
        nc.vector.tensor_copy(o_sb[:], psf[:])
        nc.sync.dma_start(out=c.out_d[b, e * 128 : (e + 1) * 128, ts], in_=o_sb[:])


_PROGRAM = None


def _program():
    global _PROGRAM
    if _PROGRAM is None:
        _PROGRAM = _build_program()
    return _PROGRAM


def _rope_tables():
    f = np.arange(32, dtype=np.float64)
    inv = ROPE_BASE ** (-2.0 * f / HD)
    t = np.arange(L, dtype=np.float64)
    ang = np.outer(inv, t)  # [32, L]
    cosT = np.cos(ang)
    sinT = np.sin(ang)
    cc = np.tile(cosT, (4, 1)).astype(np.float32)  # [128, L]
    ssw = np.concatenate([sinT, -sinT, sinT, -sinT], axis=0).astype(np.float32)
    return cc, ssw


def _prep_in_maps(x, W_qkv, W_proj):
    xt = np.ascontiguousarray(x.reshape(TOK, D).T).astype(np.float16)
    cc, ssw = _rope_tables()
    scale = HD**-0.5

    evens = np.arange(0, HD, 2)
    odds = np.arange(1, HD, 2)
    in_maps = []
    for c in range(NCORES):
        h0, h1 = HPC * c, HPC * c + 1
        rows_pair = np.concatenate(
            [h0 * HD + evens, h0 * HD + odds, h1 * HD + evens, h1 * HD + odds]
        )
        wq = (W_qkv[rows_pair, :].astype(np.float64) * scale).T  # [D, 128]
        wk = W_qkv[D + rows_pair, :].T  # [D, 128]
        wqk = np.concatenate([wq, wk], axis=1).astype(np.float16)
        rows_v = np.concatenate(
            [2 * D + h0 * HD + np.arange(HD), 2 * D + h1 * HD + np.arange(HD)]
        )
        wv = np.ascontiguousarray(W_qkv[rows_v, :].T).astype(np.float16)  # [D, 128]
        d_rows = np.concatenate([h0 * HD + np.arange(HD), h1 * HD + np.arange(HD)])
        wp = np.ascontiguousarray(W_proj[:, d_rows].T).astype(np.float16)  # [128, D]
        in_maps.append(
            {"xt": xt, "wqk": wqk, "wv": wv, "wp": wp, "cc": cc, "ssw": ssw}
        )
    return in_maps


def run(x, W_qkv, W_proj, trace=False):
    nc = _program()
    in_maps = _prep_in_maps(np.asarray(x), np.asarray(W_qkv), np.asarray(W_proj))
    res = run_bass_kernel_spmd(
        nc, in_maps, core_ids=list(range(NCORES)), trace=trace
    )
    acc = res.results[0]["out"].astype(np.float64)
    for c in range(1, NCORES):
        acc += res.results[c]["out"]
    full = np.transpose(acc, (0, 2, 1)).astype(np.float32)  # [B, L, D]
    return full, res


def kernel(x, W_qkv, W_proj):
    out, _ = run(x, W_qkv, W_proj, trace=False)
    return out


# revision 18
# speedup vs baseline: 1.8532x; 1.0540x over previous
"""Multi-head attention with RoPE on 8 Trainium2 NeuronCores.

Problem: B=4, L=2048, D=1024, H=16 heads of dim 64, fp32, full (non-causal)
softmax attention with concatenated-halves RoPE on q and k.

Sharding: tensor-parallel over heads. Each of the 8 cores owns 2 heads:
 - computes q/k/v projections for its heads only (W_qkv column slice),
 - runs attention for its 2 heads x 4 batches,
 - computes a rank-128 partial of the output projection (W_proj row slice).
The host sums the 8 partial outputs (the only cross-core reduction).

On-core layout choices:
 - q, k are produced FEATURE-major ([head_dim, tokens]) directly by the QKV
   GEMM (weights pre-transposed/permuted on host), so the QK^T matmul needs
   no transposes. RoPE's even/odd feature split is pre-applied as a row
   permutation of W_q/W_k, so RoPE becomes 3 full-width DVE ops plus a
   32-partition-block swap done with SBUF->SBUF DMA.
 - scores are computed TRANSPOSED ([k_tokens, q_tokens]); softmax exp runs on
   ACT (scale folded into W_q on the host); the denominator comes free as an
   extra all-ones column appended to v in the p@v matmul.
 - v is produced feature-major then PE-transposed to token-major.
 - matmul operands are fp16 (PE streams 1 cycle/row vs 2 for fp32/fp32r);
   all accumulation stays fp32 in PSUM, softmax/rope/normalization math is
   fp32. Measured end-to-end relative error ~8e-4.
 - emission is software-pipelined: phase1 of batch b+1 and the output
   projection of batch b are emitted between the attention chunks of batch
   b so the Tile scheduler interleaves them into ACT-bound gaps.
"""

import sys

for _p in ("/opt/trn_rl_repo",):
    if _p not in sys.path:
        sys.path.insert(0, _p)

import numpy as np
import concourse.bass as bass
import concourse.mybir as mybir
from concourse import bacc
from concourse.tile import TileContext
from concourse.bass_utils import run_bass_kernel_spmd
from concourse.masks import make_identity

F32 = mybir.dt.float32
F16 = mybir.dt.float16

B, L, D = 4, 2048, 1024
H, HD = 16, 64
NCORES = 8
HPC = H // NCORES  # 2 heads per core
TOK = B * L
BLK = 512  # gemm moving-dim block
QBLK = 512  # attention query block (one PSUM bank of fp32 output)
NBLK = L // BLK  # 4
NQB = L // QBLK  # 4
KT = D // 128  # 8 contraction tiles for the qkv projection
NKJ = L // 128  # 16 key tiles per batch
ROPE_BASE = 10000.0

Exp = mybir.ActivationFunctionType.Exp


class _Ctx:
    pass


def _build_program():
    nc = bacc.Bacc("TRN2", target_bir_lowering=False, debug=False)

    c = _Ctx()
    c.nc = nc
    c.xt_d = nc.dram_tensor("xt", [D, TOK], F16, kind="ExternalInput")
    c.wqk_d = nc.dram_tensor("wqk", [D, 256], F16, kind="ExternalInput")
    c.wv_d = nc.dram_tensor("wv", [D, 128], F16, kind="ExternalInput")
    c.wp_d = nc.dram_tensor("wp", [128, D], F16, kind="ExternalInput")
    c.cc_d = nc.dram_tensor("cc", [128, L], F32, kind="ExternalInput")
    c.ssw_d = nc.dram_tensor("ssw", [128, L], F32, kind="ExternalInput")
    c.out_d = nc.dram_tensor("out", [B, D, L], F32, kind="ExternalOutput")

    with TileContext(nc) as tc:
        with (
            tc.tile_pool(name="singles", bufs=1) as singles,
            tc.tile_pool(name="xin", bufs=2) as xin,
            tc.tile_pool(name="batch", bufs=2) as batch,
            tc.tile_pool(name="rope", bufs=2) as rope,
            tc.tile_pool(name="pexp", bufs=4) as pexp,
            tc.tile_pool(name="norm", bufs=4) as norm,
            tc.tile_pool(name="outp", bufs=4) as outp,
            tc.tile_pool(name="ps_g", bufs=2, space="PSUM") as ps_g,
            tc.tile_pool(name="ps_s", bufs=2, space="PSUM") as ps_s,
            tc.tile_pool(name="ps_o", bufs=2, space="PSUM") as ps_o,
        ):
            c.xin, c.batch, c.rope = xin, batch, rope
            c.pexp, c.norm, c.outp = pexp, norm, outp
            c.ps_g, c.ps_s, c.ps_o = ps_g, ps_s, ps_o

            # resident weights / tables
            c.wqk_sb = singles.tile([128, KT, 256], F16, tag="wqk")
            nc.sync.dma_start(
                out=c.wqk_sb[:], in_=c.wqk_d[:, :].rearrange("(k p) e -> p k e", p=128)
            )
            c.wv_sb = singles.tile([128, KT, 128], F16, tag="wv")
            nc.sync.dma_start(
                out=c.wv_sb[:], in_=c.wv_d[:, :].rearrange("(k p) e -> p k e", p=128)
            )
            c.wp_sb = singles.tile([128, D], F16, tag="wp")
            nc.sync.dma_start(out=c.wp_sb[:], in_=c.wp_d[:, :])
            c.cc_sb = singles.tile([128, L], F32, tag="cc")
            nc.sync.dma_start(out=c.cc_sb[:], in_=c.cc_d[:, :])
            c.ssw_sb = singles.tile([128, L], F32, tag="ssw")
            nc.sync.dma_start(out=c.ssw_sb[:], in_=c.ssw_d[:, :])
            c.ident = singles.tile([128, 128], F16, tag="ident")
            make_identity(nc, c.ident[:])

            c.bt = {}

            # software-pipelined emission
            for blk in range(NBLK):
                _phase1_chunk(c, 0, blk)
            _vtrans(c, 0)
            for b in range(B):
                for qi in range(NQB):
                    _phase2_chunk(c, b, qi)
                    if b + 1 < B:
                        _phase1_chunk(c, b + 1, qi)
                        if qi == NQB - 1:
                            _vtrans(c, b + 1)
                    if qi >= 1:
                        _phase3_chunk(c, b, qi - 1)
                _phase3_chunk(c, b, NQB - 1)

    nc.compile()
    return nc


def _tiles(c, b):
    if b not in c.bt:
        t = _Ctx()
        t.q_ro = c.batch.tile([128, L], F16, tag="qro")
        t.k_ro = c.batch.tile([128, L], F16, tag="kro")
        t.v_fm = c.batch.tile([128, L], F16, tag="vfm")
        t.v0 = c.batch.tile([128, NKJ, 65], F16, tag="v0")
        t.v1 = c.batch.tile([128, NKJ, 65], F16, tag="v1")
        t.ao = c.batch.tile([128, L], F16, tag="ao")
        t.x_t = None
        c.bt[b] = t
    return c.bt[b]


def _phase1_chunk(c, b, blk):
    nc = c.nc
    t = _tiles(c, b)
    ts = slice(blk * BLK, (blk + 1) * BLK)
    if blk == 0:
        t.x_t = c.xin.tile([128, KT, L], F16, tag="x")
        nc.sync.dma_start(
            out=t.x_t[:],
            in_=c.xt_d[:, b * L : (b + 1) * L].rearrange("(k p) t -> p k t", p=128),
        )
    for wcol, dst in ((0, t.q_ro), (128, t.k_ro)):
        ps = c.ps_g.tile([128, BLK], F32, tag="g")
        for kd in range(KT):
            nc.tensor.matmul(
                ps[:],
                c.wqk_sb[:, kd, wcol : wcol + 128],
                t.x_t[:, kd, ts],
                start=(kd == 0),
                stop=(kd == KT - 1),
            )
        tmp_c = c.rope.tile([128, BLK], F32, tag="tc")
        nc.vector.tensor_mul(tmp_c[:], ps[:], c.cc_sb[:, ts])
        tmp_s = c.rope.tile([128, BLK], F32, tag="tsn")
        nc.vector.tensor_mul(tmp_s[:], ps[:], c.ssw_sb[:, ts])
        tmp_w = c.rope.tile([128, BLK], F32, tag="tw")
        for a, bb in ((0, 32), (32, 0), (64, 96), (96, 64)):
            nc.sync.dma_start(out=tmp_w[a : a + 32, :], in_=tmp_s[bb : bb + 32, :])
        nc.vector.tensor_add(dst[:, ts], tmp_c[:], tmp_w[:])

    psv = c.ps_g.tile([128, BLK], F32, tag="g")
    for kd in range(KT):
        nc.tensor.matmul(
            psv[:],
            c.wv_sb[:, kd, :],
            t.x_t[:, kd, ts],
            start=(kd == 0),
            stop=(kd == KT - 1),
        )
    nc.scalar.copy(t.v_fm[:, ts], psv[:])


def _vtrans(c, b):
    nc = c.nc
    t = _tiles(c, b)
    nc.vector.memset(t.v0[:, :, 64], 1.0)
    nc.vector.memset(t.v1[:, :, 64], 1.0)
    for tt in range(NKJ):
        pst = c.ps_g.tile([128, 128], F16, tag="g")
        nc.tensor.transpose(pst[:], t.v_fm[:, tt * 128 : (tt + 1) * 128], c.ident[:])
        nc.vector.tensor_copy(t.v0[:, tt, 0:64], pst[:, 0:64])
        nc.vector.tensor_copy(t.v1[:, tt, 0:64], pst[:, 64:128])


def _phase2_chunk(c, b, qi):
    nc = c.nc
    t = _tiles(c, b)
    qs = slice(qi * QBLK, (qi + 1) * QBLK)
    o0 = c.ps_o.tile([65, QBLK], F32, tag="o")
    o1 = c.ps_o.tile([65, QBLK], F32, tag="o")
    for kj in range(NKJ):
        ks = slice(kj * 128, (kj + 1) * 128)
        s_ps = c.ps_s.tile([128, 2 * QBLK], F32, tag="s")
        nc.tensor.matmul(
            s_ps[:, 0:QBLK], t.k_ro[0:64, ks], t.q_ro[0:64, qs],
            start=True, stop=True,
        )
        nc.tensor.matmul(
            s_ps[:, QBLK : 2 * QBLK],
            t.k_ro[64:128, ks],
            t.q_ro[64:128, qs],
            start=True,
            stop=True,
            tile_position=(64, 0),
        )
        p = c.pexp.tile([128, 2 * QBLK], F16, tag="p")
        nc.scalar.activation(p[:], s_ps[:], Exp)
        nc.tensor.matmul(
            o0[:], t.v0[:, kj, :], p[:, 0:QBLK],
            start=(kj == 0), stop=(kj == NKJ - 1),
        )
        nc.tensor.matmul(
            o1[:], t.v1[:, kj, :], p[:, QBLK : 2 * QBLK],
            start=(kj == 0), stop=(kj == NKJ - 1),
        )
    # early copies release the o-psum banks; recip/broadcast/multiply run
    # off the PE critical path.
    rb_full = c.norm.tile([128, QBLK], F32, tag="rbf")
    for o_ps, base in ((o0, 0), (o1, 64)):
        nc.vector.tensor_copy(t.ao[base : base + 64, qs], o_ps[0:64, :])
        stg = c.norm.tile([1, QBLK], F32, tag="stg")
        nc.scalar.copy(stg[:], o_ps[64:65, :])
        r = c.norm.tile([1, QBLK], F32, tag="r")
        nc.vector.reciprocal_approx_fast(r[:], stg[:])
        if base == 0:
            nc.gpsimd.partition_broadcast(rb_full[0:64, :], r[:])
        else:
            rb1 = c.norm.tile([64, QBLK], F32, tag="rb")
            nc.gpsimd.partition_broadcast(rb1[:], r[:])
            nc.vector.tensor_copy(rb_full[64:128, :], rb1[:])
    nc.vector.tensor_mul(t.ao[:, qs], t.ao[:, qs], rb_full[:])


def _phase3_chunk(c, b, blk):
    nc = c.nc
    t = _tiles(c, b)
    ts = slice(blk * BLK, (blk + 1) * BLK)
    for e in range(D // 128):
        psf = c.ps_g.tile([128, BLK], F32, tag="g")
        nc.tensor.matmul(
            psf[:],
            c.wp_sb[:, e * 128 : (e + 1) * 128],
            t.ao[:, ts],
            start=True,
            stop=True,
        )
        o_sb = c.outp.tile([128, BLK], F32, tag="os")
        nc.vector.tensor_copy(o_sb[:], psf[:])
        nc.sync.dma_start(out=c.out_d[b, e * 128 : (e + 1) * 128, ts], in_=o_sb[:])


_PROGRAM = None


def _program():
    global _PROGRAM
    if _PROGRAM is None:
        _PROGRAM = _build_program()
    return _PROGRAM


def _rope_tables():
    f = np.arange(32, dtype=np.float64)
    inv = ROPE_BASE ** (-2.0 * f / HD)
    t = np.arange(L, dtype=np.float64)
    ang = np.outer(inv, t)  # [32, L]
    cosT = np.cos(ang)
    sinT = np.sin(ang)
    cc = np.tile(cosT, (4, 1)).astype(np.float32)  # [128, L]
    ssw = np.concatenate([sinT, -sinT, sinT, -sinT], axis=0).astype(np.float32)
    return cc, ssw


def _prep_in_maps(x, W_qkv, W_proj):
    xt = np.ascontiguousarray(x.reshape(TOK, D).T).astype(np.float16)
    cc, ssw = _rope_tables()
    scale = HD**-0.5

    evens = np.arange(0, HD, 2)
    odds = np.arange(1, HD, 2)
    in_maps = []
    for c in range(NCORES):
        h0, h1 = HPC * c, HPC * c + 1
        rows_pair = np.concatenate(
            [h0 * HD + evens, h0 * HD + odds, h1 * HD + evens, h1 * HD + odds]
        )
        wq = (W_qkv[rows_pair, :].astype(np.float64) * scale).T  # [D, 128]
        wk = W_qkv[D + rows_pair, :].T  # [D, 128]
        wqk = np.concatenate([wq, wk], axis=1).astype(np.float16)
        rows_v = np.concatenate(
            [2 * D + h0 * HD + np.arange(HD), 2 * D + h1 * HD + np.arange(HD)]
        )
        wv = np.ascontiguousarray(W_qkv[rows_v, :].T).astype(np.float16)  # [D, 128]
        d_rows = np.concatenate([h0 * HD + np.arange(HD), h1 * HD + np.arange(HD)])
        wp = np.ascontiguousarray(W_proj[:, d_rows].T).astype(np.float16)  # [128, D]
        in_maps.append(
            {"xt": xt, "wqk": wqk, "wv": wv, "wp": wp, "cc": cc, "ssw": ssw}
        )
    return in_maps


def run(x, W_qkv, W_proj, trace=False):
    nc = _program()
    in_maps = _prep_in_maps(np.asarray(x), np.asarray(W_qkv), np.asarray(W_proj))
    res = run_bass_kernel_spmd(
        nc, in_maps, core_ids=list(range(NCORES)), trace=trace
    )
    acc = res.results[0]["out"].astype(np.float64)
    for c in range(1, NCORES):
        acc += res.results[c]["out"]
    full = np.transpose(acc, (0, 2, 1)).astype(np.float32)  # [B, L, D]
    return full, res


def kernel(x, W_qkv, W_proj):
    out, _ = run(x, W_qkv, W_proj, trace=False)
    return out
